# revision 1
# baseline (speedup 1.0000x reference)
"""Cubic B-spline evaluation (uniform knots) on 8 Trainium2 NeuronCores.

v2: j = 2q + r split.  On segment j the spline is a cubic in v = x - 2q:
  out = HC(v) + r * HD(v),  HC = sum_k c_k[q] v^k,  HD = sum_k d_k[q] v^k
with 32-entry tables c, d (host-derived from coefs).  Table lookups become
step sums over 32 thresholds 1{q >= i} = 1{j >= 2i}, built as a K=5 bf16
matmul over 4 point-slots packed into 128 partitions (32 rows each), an
indicator pass (ScalarE Sign / VectorE is_ge), and a contraction with bf16
hi+lo difference weights.  Coefficient octets stream through DRAM scratch
into pointwise layout; a dual Horner finishes.

Layout (per core, N = 131072 = 4 slots x 32768):
  pointwise: x_pw[p, f] = x[1024 p + f]; p = 32 s + q, q = 2 t + b
  tiles: 64 x 512 cols; chunk t in [0,16) x tau in [0,4); tg = 4t + tau;
         q = tg//2, h = tg%2; unit U = q//2 = t, e = q%2
  g_all[p, cd, k, h, c]: coef k of table cd for point (p, f = 512 h + c)
"""

import sys

sys.path.insert(0, "/opt/trn_rl_repo")

import numpy as np

N_TOTAL = 1_048_576
N_CORES = 8
N = N_TOTAL // N_CORES  # 131072 points per core
P = 128
COLS = N // P  # 1024
TW = 512
NCHUNK = 16
TPC = 4
CH = TPC * TW  # 4096
NSLOT = 4
SLOTN = N // NSLOT  # 32768


def _tables(coefs: np.ndarray):
    import ml_dtypes

    c = np.zeros(67, np.float64)
    c[3:] = np.asarray(coefs, np.float64)
    jj = np.arange(64)
    a0 = (c[jj] + 4 * c[jj + 1] + c[jj + 2]) / 6
    a1 = (c[jj + 2] - c[jj]) / 2
    a2 = (c[jj] - 2 * c[jj + 1] + c[jj + 2]) / 2
    a3 = (c[jj + 3] - c[jj] + 3 * c[jj + 1] - 3 * c[jj + 2]) / 6
    A = np.stack([a0, a1, a2, a3], 1)  # [64, 4] coeffs in u = x - j

    # rebase odd segments to v = u + 1 (v = x - 2q)
    B = A.copy()
    r1 = jj % 2 == 1
    B[r1, 0] = A[r1, 0] - A[r1, 1] + A[r1, 2] - A[r1, 3]
    B[r1, 1] = A[r1, 1] - 2 * A[r1, 2] + 3 * A[r1, 3]
    B[r1, 2] = A[r1, 2] - 3 * A[r1, 3]
    B[r1, 3] = A[r1, 3]
    C = B[0::2]  # [32, 4]
    D = B[1::2] - B[0::2]  # [32, 4]

    # halved step-difference weights (unified sign/{0,2} convention)
    WC = C.copy()
    WC[1:] -= C[:-1]
    WD = D.copy()
    WD[1:] -= D[:-1]
    Wp = np.concatenate([WC, WD], 1) * 0.5  # [32, 8]: col 4 cd + k
    gamma_k = Wp.sum(0).astype(np.float32)  # [8]

    # MM1 lhsT [5, 128]: col m = 32 s + i -> psum = jf_s - thr_i
    w1 = np.zeros((5, 128), np.float64)
    thr = np.empty(32)
    thr[0] = -1.0
    thr[1:] = 2.0 * np.arange(1, 32) - 0.5
    for s in range(4):
        w1[s, 32 * s : 32 * s + 32] = 1.0
        w1[4, 32 * s : 32 * s + 32] = -thr
    # MM2 lhsT [128, 32]: row m = 32 s' + i, col 8 s + 4 cd + k
    w2 = np.zeros((128, 32), np.float64)
    for s in range(4):
        w2[32 * s : 32 * s + 32, 8 * s : 8 * s + 8] = Wp
    bf = ml_dtypes.bfloat16
    w2hi = w2.astype(bf)
    w2lo = (w2 - w2hi.astype(np.float64)).astype(bf)
    return A, w1.astype(bf), (w2hi, w2lo), gamma_k


def _eng_of(t: int, b: int) -> str:
    return "act" if (4 * t + b) % 5 < 3 else "dve"


def _gamma_vec(gamma_k: np.ndarray) -> np.ndarray:
    g = np.zeros((P, 8), np.float32)
    for p in range(P):
        q = p % 32  # q = 2 t + b  (TPC = 4: two pairs per chunk)
        if _eng_of(q // 2, q % 2) == "act":
            g[p] = gamma_k
    return g


_PROG_CACHE: dict = {}


def _build_program():
    import concourse.bacc as bacc
    import concourse.mybir as mybir
    from concourse.tile import TileContext

    f32 = mybir.dt.float32
    bf16 = mybir.dt.bfloat16
    Alu = mybir.AluOpType

    nc = bacc.Bacc("TRN2", debug=False)

    x_dram = nc.dram_tensor("x", [N], f32, kind="ExternalInput")
    w1_dram = nc.dram_tensor("w1", [5, P], bf16, kind="ExternalInput")
    w2hi_dram = nc.dram_tensor("w2hi", [P, 32], bf16, kind="ExternalInput")
    w2lo_dram = nc.dram_tensor("w2lo", [P, 32], bf16, kind="ExternalInput")
    g_dram = nc.dram_tensor("gamma", [P, 8], f32, kind="ExternalInput")
    ones_dram = nc.dram_tensor("ones", [1, CH], bf16, kind="ExternalInput")
    out_dram = nc.dram_tensor("out", [N], f32, kind="ExternalOutput")
    jf_dram = nc.dram_tensor("jf_scratch", [N], bf16, kind="Internal")
    g_dram_s = nc.dram_tensor("g_scratch", [16, 32, 4 * TW], f32, kind="Internal")

    x_view = x_dram.ap().rearrange("(p f) -> p f", p=P)
    out_view = out_dram.ap().rearrange("(p f) -> p f", p=P)

    with TileContext(nc) as tc:
        with (
            tc.tile_pool(name="const", bufs=1) as cpool,
            tc.tile_pool(name="pw", bufs=1) as pw,
            tc.tile_pool(name="tmp", bufs=6) as tmp,
            tc.tile_pool(name="sind", bufs=1) as spool,
            tc.tile_pool(name="gcp", bufs=1) as gcpool,
            tc.tile_pool(name="psum1", bufs=1, space="PSUM") as pp1,
            tc.tile_pool(name="psum2", bufs=1, space="PSUM") as pp2,
        ):
            # ---- constants ----
            w1_sb = cpool.tile([5, P], bf16, tag="w1")
            nc.sync.dma_start(out=w1_sb[:], in_=w1_dram.ap())
            w2hi_sb = cpool.tile([P, 32], bf16, tag="w2hi")
            nc.sync.dma_start(out=w2hi_sb[:], in_=w2hi_dram.ap())
            w2lo_sb = cpool.tile([P, 32], bf16, tag="w2lo")
            nc.sync.dma_start(out=w2lo_sb[:], in_=w2lo_dram.ap())
            gam_sb = cpool.tile([P, 8], f32, tag="gam")
            nc.sync.dma_start(out=gam_sb[:], in_=g_dram.ap())
            j_bufs = []
            for bi in range(3):
                jb = cpool.tile([5, CH], bf16, tag=f"jbuf{bi}", name=f"jbuf{bi}")
                nc.sync.dma_start(out=jb[4:5, :], in_=ones_dram.ap())
                j_bufs.append(jb)

            ps1_bufs = [
                pp1.tile([P, TW], f32, tag=f"s1_{i}", name=f"ps1f{i}")
                for i in range(4)
            ]
            ps2_bufs = [
                pp2.tile([32, 2 * TW], f32, tag=f"s2_{i}", name=f"ps2f{i}")
                for i in range(2)
            ]
            s_bufs = [
                spool.tile([P, TW], bf16, tag=f"sb_{i}", name=f"sbf{i}")
                for i in range(8)
            ]
            gcp_full = [
                gcpool.tile([32, 4 * TW], f32, tag=f"gc_{i}", name=f"gcpf{i}")
                for i in range(2)
            ]

            # dummies: absorb constant-load DMA sems into the PE vector clock
            pdum = ps1_bufs[0]
            nc.tensor.matmul(
                out=pdum[:, 0:8], lhsT=w1_sb[:], rhs=w1_sb[:, 0:8],
                start=True, stop=True,
            )
            nc.tensor.matmul(
                out=pdum[0:32, 0:8], lhsT=w2hi_sb[:], rhs=w2hi_sb[:, 0:8],
                start=True, stop=True,
            )
            nc.tensor.matmul(
                out=pdum[0:32, 0:8], lhsT=w2lo_sb[:], rhs=w2lo_sb[:, 0:8],
                start=True, stop=True,
            )

            # ---- pointwise prep: jf = floor(x), qf = floor(x/2) ----
            x_pw = pw.tile([P, COLS], f32, tag="x")
            nc.sync.dma_start(out=x_pw[:], in_=x_view)
            jf_pw = pw.tile([P, COLS], bf16, tag="jf")
            r_pw = pw.tile([P, COLS], f32, tag="r")
            nc.vector.tensor_scalar(
                r_pw[:], x_pw[:], 8388608.0, -8388608.0, Alu.add, Alu.add
            )
            d_pw = pw.tile([P, COLS], f32, tag="d")
            nc.vector.tensor_tensor(
                out=d_pw[:], in0=r_pw[:], in1=x_pw[:], op=Alu.is_gt
            )
            nc.vector.tensor_tensor(
                out=jf_pw[:], in0=r_pw[:], in1=d_pw[:], op=Alu.subtract
            )
            hx_pw = pw.tile([P, COLS], f32, tag="hx")
            nc.scalar.mul(hx_pw[:], x_pw[:], 0.5)
            t2_pw = pw.tile([P, COLS], f32, tag="t2")
            nc.vector.tensor_scalar(
                t2_pw[:], hx_pw[:], 8388608.0, -8388608.0, Alu.add, Alu.add
            )
            d2_pw = pw.tile([P, COLS], f32, tag="d2")
            nc.vector.tensor_tensor(
                out=d2_pw[:], in0=t2_pw[:], in1=hx_pw[:], op=Alu.is_gt
            )
            qf_pw = pw.tile([P, COLS], f32, tag="qf")
            nc.vector.tensor_tensor(
                out=qf_pw[:], in0=t2_pw[:], in1=d2_pw[:], op=Alu.subtract
            )
            v_pw = pw.tile([P, COLS], f32, tag="v")
            nc.vector.scalar_tensor_tensor(
                v_pw[:], qf_pw[:], -2.0, x_pw[:], Alu.mult, Alu.add
            )
            # r = jf - 2 qf  (0/1)
            rr_pw = pw.tile([P, COLS], f32, tag="rr")
            nc.vector.scalar_tensor_tensor(
                rr_pw[:], qf_pw[:], -2.0, jf_pw[:], Alu.mult, Alu.add
            )

            nc.sync.dma_start(
                out=jf_dram.ap().rearrange("(p f) -> p f", p=P), in_=jf_pw[:]
            )
            jf_view = jf_dram.ap().rearrange(
                "(s t cc) -> s t cc", s=NSLOT, t=NCHUNK
            )

            g_all = pw.tile([P, 2, 4, 2, TW], f32, tag="gall")
            g_view = g_dram_s.ap().rearrange(
                "u m (e h c) -> m u e (h c)", e=2, h=2
            )

            # ---- chunk loop ----
            for t in range(NCHUNK):
                if t == 9:
                    # first half of the units is stored; stream those loads
                    for cd in range(2):
                        for k in range(4):
                            for s in range(4):
                                nc.sync.dma_start(
                                    out=g_all[32 * s : 32 * s + 16, cd, k, :, :],
                                    in_=g_view[8 * s + 4 * cd + k, 0:8],
                                )
                j_pk = j_bufs[t % 3]
                nc.sync.dma_start(out=j_pk[0:4, :], in_=jf_view[:, t])
                # consolidator for the jf-load semaphore
                nc.tensor.matmul(
                    out=ps1_bufs[0][:, 0:8], lhsT=w1_sb[0:4, :],
                    rhs=j_pk[0:4, 0:8], start=True, stop=True,
                )
                for tau in range(TPC):
                    b, h = tau // 2, tau % 2
                    tg = TPC * t + tau
                    q = tg // 2  # = 4 t + b
                    ps1 = ps1_bufs[tg % 4]
                    nc.tensor.matmul(
                        out=ps1[:],
                        lhsT=w1_sb[:],
                        rhs=j_pk[:, tau * TW : (tau + 1) * TW],
                        start=True,
                        stop=True,
                    )
                    s_sb = s_bufs[tg % 8]
                    if _eng_of(t, b) == "act":
                        nc.scalar.sign(s_sb[:], ps1[:])  # {-1, +1}
                    else:
                        nc.vector.tensor_scalar(
                            s_sb[:], ps1[:], 0.0, 2.0, Alu.is_ge, Alu.mult
                        )  # {0, 2}
                    ps2 = ps2_bufs[q % 2]
                    nc.tensor.matmul(
                        out=ps2[:, h * TW : (h + 1) * TW],
                        lhsT=w2hi_sb[:], rhs=s_sb[:],
                        start=True, stop=False,
                    )
                    nc.tensor.matmul(
                        out=ps2[:, h * TW : (h + 1) * TW],
                        lhsT=w2lo_sb[:], rhs=s_sb[:],
                        start=False, stop=True,
                    )
                    if h == 1:
                        gcp = gcp_full[(q // 2) % 2]
                        dstc = gcp[:, (q % 2) * 2 * TW : (q % 2 + 1) * 2 * TW]
                        if (q * 3) % 5 < 3:
                            nc.scalar.copy(out=dstc, in_=ps2[:])
                        else:
                            nc.vector.tensor_copy(out=dstc, in_=ps2[:])
                    if tau % 4 == 3:
                        U = tg // 4
                        nc.gpsimd.dma_start(
                            out=g_dram_s.ap()[U], in_=gcp_full[U % 2][:]
                        )

            # ---- remaining G loads (u >= 8) ----
            for cd in range(2):
                for k in range(4):
                    for s in range(4):
                        nc.sync.dma_start(
                            out=g_all[32 * s + 16 : 32 * s + 32, cd, k, :, :],
                            in_=g_view[8 * s + 4 * cd + k, 8:16],
                        )

            # ---- dual Horner: out = HC(v) + r * HD(v), + gamma on ACT rows --
            v2_pw = pw.tile([P, COLS], f32, tag="v2")
            nc.scalar.square(v2_pw[:], v_pw[:])
            hres = []
            for cd in range(2):
                gk = [
                    g_all[:, cd, k].rearrange("p h c -> p (h c)")
                    for k in range(4)
                ]
                g2c = tmp.tile([P, COLS], f32, tag="ta", name=f"g2c{cd}")
                nc.vector.tensor_scalar(
                    g2c[:], gk[2], gam_sb[:, 4 * cd + 2 : 4 * cd + 3], None,
                    Alu.add,
                )
                g3c = tmp.tile([P, COLS], f32, tag="tb", name=f"g3c{cd}")
                nc.vector.tensor_scalar(
                    g3c[:], gk[3], gam_sb[:, 4 * cd + 3 : 4 * cd + 4], None,
                    Alu.add,
                )
                v1t = tmp.tile([P, COLS], f32, tag="tc", name=f"v1t{cd}")
                nc.vector.tensor_tensor(
                    out=v1t[:], in0=g2c[:], in1=v2_pw[:], op=Alu.mult
                )
                v2t = tmp.tile([P, COLS], f32, tag="td", name=f"v2t{cd}")
                nc.vector.tensor_tensor(
                    out=v2t[:], in0=g3c[:], in1=v2_pw[:], op=Alu.mult
                )
                pacc = tmp.tile([P, COLS], f32, tag="ta", name=f"pacc{cd}")
                nc.vector.scalar_tensor_tensor(
                    pacc[:], v1t[:], gam_sb[:, 4 * cd : 4 * cd + 1], gk[0],
                    Alu.add, Alu.add,
                )
                qacc = tmp.tile([P, COLS], f32, tag="tb", name=f"qacc{cd}")
                nc.vector.scalar_tensor_tensor(
                    qacc[:], v2t[:], gam_sb[:, 4 * cd + 1 : 4 * cd + 2], gk[1],
                    Alu.add, Alu.add,
                )
                v3t = tmp.tile([P, COLS], f32, tag="tc", name=f"v3t{cd}")
                nc.vector.tensor_tensor(
                    out=v3t[:], in0=qacc[:], in1=v_pw[:], op=Alu.mult
                )
                hr = tmp.tile([P, COLS], f32, tag="td", name=f"hr{cd}")
                nc.vector.tensor_tensor(
                    out=hr[:], in0=pacc[:], in1=v3t[:], op=Alu.add
                )
                hres.append(hr)
            rd = tmp.tile([P, COLS], f32, tag="ta", name="rd")
            nc.vector.tensor_tensor(
                out=rd[:], in0=hres[1][:], in1=rr_pw[:], op=Alu.mult
            )
            res = tmp.tile([P, COLS], f32, tag="tb", name="res")
            nc.vector.tensor_tensor(
                out=res[:], in0=hres[0][:], in1=rd[:], op=Alu.add
            )
            nc.sync.dma_start(out=out_view, in_=res[:])

    nc.compile()
    return nc


def get_program():
    if "prog" not in _PROG_CACHE:
        _PROG_CACHE["prog"] = _build_program()
    return _PROG_CACHE["prog"]


def make_in_maps(x: np.ndarray, coefs: np.ndarray):
    import ml_dtypes

    _, w1, (w2hi, w2lo), gamma_k = _tables(coefs)
    gvec = _gamma_vec(gamma_k)
    shards = np.asarray(x, np.float32).reshape(N_CORES, N)
    ones = np.ones((1, CH), ml_dtypes.bfloat16)
    return [
        {
            "x": shards[i].copy(),
            "w1": w1,
            "w2hi": w2hi,
            "w2lo": w2lo,
            "gamma": gvec,
            "ones": ones,
        }
        for i in range(N_CORES)
    ]


def kernel(x, coefs, knot_vector=None, _trace: bool = False):
    from concourse.bass_utils import run_bass_kernel_spmd

    nc = get_program()
    in_maps = make_in_maps(x, coefs)
    res = run_bass_kernel_spmd(nc, in_maps, list(range(N_CORES)), trace=_trace)
    out = np.concatenate([r["out"] for r in res.results])
    if _trace:
        return out, res
    return out



# revision 16
# speedup vs baseline: 1.0770x; 1.0770x over previous
"""Cubic B-spline evaluation (uniform knots) on 8 Trainium2 NeuronCores.

v2: j = 2q + r split.  On segment j the spline is a cubic in v = x - 2q:
  out = HC(v) + r * HD(v),  HC = sum_k c_k[q] v^k,  HD = sum_k d_k[q] v^k
with 32-entry tables c, d (host-derived from coefs).  Table lookups become
step sums over 32 thresholds 1{q >= i} = 1{j >= 2i}, built as a K=5 bf16
matmul over 4 point-slots packed into 128 partitions (32 rows each), an
indicator pass (ScalarE Sign / VectorE is_ge), and a contraction with bf16
hi+lo difference weights.  Coefficient octets stream through DRAM scratch
into pointwise layout; a dual Horner finishes.

Layout (per core, N = 131072 = 4 slots x 32768):
  pointwise: x_pw[p, f] = x[1024 p + f]; p = 32 s + q, q = 2 t + b
  tiles: 64 x 512 cols; chunk t in [0,16) x tau in [0,4); tg = 4t + tau;
         q = tg//2, h = tg%2; unit U = q//2 = t, e = q%2
  g_all[p, cd, k, h, c]: coef k of table cd for point (p, f = 512 h + c)
"""

import sys

sys.path.insert(0, "/opt/trn_rl_repo")

import numpy as np

N_TOTAL = 1_048_576
N_CORES = 8
N = N_TOTAL // N_CORES  # 131072 points per core
P = 128
COLS = N // P  # 1024
TW = 512
NCHUNK = 16
TPC = 4
CH = TPC * TW  # 4096
NSLOT = 4
SLOTN = N // NSLOT  # 32768


def _tables(coefs: np.ndarray):
    import ml_dtypes

    c = np.zeros(67, np.float64)
    c[3:] = np.asarray(coefs, np.float64)
    jj = np.arange(64)
    a0 = (c[jj] + 4 * c[jj + 1] + c[jj + 2]) / 6
    a1 = (c[jj + 2] - c[jj]) / 2
    a2 = (c[jj] - 2 * c[jj + 1] + c[jj + 2]) / 2
    a3 = (c[jj + 3] - c[jj] + 3 * c[jj + 1] - 3 * c[jj + 2]) / 6
    A = np.stack([a0, a1, a2, a3], 1)  # [64, 4] coeffs in u = x - j

    # rebase odd segments to v = u + 1 (v = x - 2q)
    B = A.copy()
    r1 = jj % 2 == 1
    B[r1, 0] = A[r1, 0] - A[r1, 1] + A[r1, 2] - A[r1, 3]
    B[r1, 1] = A[r1, 1] - 2 * A[r1, 2] + 3 * A[r1, 3]
    B[r1, 2] = A[r1, 2] - 3 * A[r1, 3]
    B[r1, 3] = A[r1, 3]
    C = B[0::2]  # [32, 4]
    D = B[1::2] - B[0::2]  # [32, 4]

    # halved step-difference weights (unified sign/{0,2} convention)
    WC = C.copy()
    WC[1:] -= C[:-1]
    WD = D.copy()
    WD[1:] -= D[:-1]
    Wp = np.concatenate([WC, WD], 1) * 0.5  # [32, 8]: col 4 cd + k
    gamma_k = Wp.sum(0).astype(np.float32)  # [8]

    # MM1 lhsT [5, 128]: col m = 32 s + i -> psum = jf_s - thr_i
    w1 = np.zeros((5, 128), np.float64)
    thr = np.empty(32)
    thr[0] = -1.0
    thr[1:] = 2.0 * np.arange(1, 32) - 0.5
    for s in range(4):
        w1[1 + s, 32 * s : 32 * s + 32] = 1.0
        w1[0, 32 * s : 32 * s + 32] = -thr
    # MM2 lhsT [128, 32]: row m = 32 s' + i, col 8 s + 4 cd + k
    w2 = np.zeros((128, 32), np.float64)
    for s in range(4):
        w2[32 * s : 32 * s + 32, 8 * s : 8 * s + 8] = Wp
    bf = ml_dtypes.bfloat16
    w2hi = w2.astype(bf)
    w2lo = (w2 - w2hi.astype(np.float64)).astype(bf)
    # pack all constants into one [128, 208] bf16 tensor:
    #   cols   0:128  rows 0:5   w1
    #   cols 128:160  w2hi, cols 160:192  w2lo
    #   cols 192:208  gamma (f32 bit-packed as bf16 pairs)
    pack = np.zeros((128, 208), bf)
    pack[0:5, 0:128] = w1.astype(bf)
    pack[:, 128:160] = w2hi
    pack[:, 160:192] = w2lo
    gvec = _gamma_vec(gamma_k)  # [128, 8] f32
    pack[:, 192:208] = gvec.astype(np.float32).view(np.uint16).view(bf)
    return A, pack


def _eng_of(t: int, b: int) -> str:
    return "act" if (4 * t + b) % 5 < 3 else "dve"


def _gamma_vec(gamma_k: np.ndarray) -> np.ndarray:
    g = np.zeros((P, 8), np.float32)
    for p in range(P):
        q = p % 32  # q = 2 t + b  (TPC = 4: two pairs per chunk)
        if _eng_of(q // 2, q % 2) == "act":
            g[p] = gamma_k
    return g


_PROG_CACHE: dict = {}


def _build_program():
    import concourse.bacc as bacc
    import concourse.mybir as mybir
    from concourse.tile import TileContext

    f32 = mybir.dt.float32
    bf16 = mybir.dt.bfloat16
    Alu = mybir.AluOpType

    nc = bacc.Bacc("TRN2", debug=False)

    x_dram = nc.dram_tensor("x", [N], f32, kind="ExternalInput")
    cpack_dram = nc.dram_tensor("cpack", [P, 208], bf16, kind="ExternalInput")
    out_dram = nc.dram_tensor("out", [N], f32, kind="ExternalOutput")
    jf_dram = nc.dram_tensor("jf_scratch", [N], bf16, kind="Internal")
    g_dram_s = nc.dram_tensor("g_scratch", [16, 32, 4 * TW], f32, kind="Internal")

    x_view = x_dram.ap().rearrange("(p f) -> p f", p=P)
    out_view = out_dram.ap().rearrange("(p f) -> p f", p=P)

    with TileContext(nc) as tc:
        with (
            tc.tile_pool(name="const", bufs=1) as cpool,
            tc.tile_pool(name="pw", bufs=1) as pw,
            tc.tile_pool(name="tmp", bufs=6) as tmp,
            tc.tile_pool(name="sind", bufs=1) as spool,
            tc.tile_pool(name="gcp", bufs=1) as gcpool,
            tc.tile_pool(name="psum1", bufs=1, space="PSUM") as pp1,
            tc.tile_pool(name="psum2", bufs=1, space="PSUM") as pp2,
        ):
            # ---- constants: one packed DMA; ones rows via memset ----
            cpk = cpool.tile([P, 208], bf16, tag="cpk")
            nc.sync.dma_start(out=cpk[:], in_=cpack_dram.ap())
            w1_sb = cpk[0:5, 0:128]
            w2hi_sb = cpk[:, 128:160]
            w2lo_sb = cpk[:, 160:192]
            gam_sb = cpk[:, 192:208].bitcast(f32)
            j_bufs = []
            for bi in range(2):
                jb = cpool.tile(
                    [5, 2 * CH], bf16, tag=f"jbuf{bi}", name=f"jbuf{bi}"
                )
                nc.vector.memset(jb[0:1, :], 1.0)
                j_bufs.append(jb)

            ps1_bufs = [
                pp1.tile([P, TW], f32, tag=f"s1_{i}", name=f"ps1f{i}")
                for i in range(4)
            ]
            ps2_bufs = [
                pp2.tile([32, 2 * TW], f32, tag=f"s2_{i}", name=f"ps2f{i}")
                for i in range(2)
            ]
            s_bufs = [
                spool.tile([P, TW], bf16, tag=f"sb_{i}", name=f"sbf{i}")
                for i in range(8)
            ]
            gcp_full = [
                gcpool.tile([32, 4 * TW], f32, tag=f"gc_{i}", name=f"gcpf{i}")
                for i in range(2)
            ]

            # dummies: absorb constant-load DMA sems into the PE vector clock
            pdum = ps1_bufs[0]
            nc.tensor.matmul(
                out=pdum[:, 0:8], lhsT=w1_sb[:], rhs=w1_sb[:, 0:8],
                start=True, stop=True,
            )
            nc.tensor.matmul(
                out=pdum[0:32, 0:8], lhsT=w2hi_sb[:], rhs=w2hi_sb[:, 0:8],
                start=True, stop=True,
            )
            nc.tensor.matmul(
                out=pdum[0:32, 0:8], lhsT=w2lo_sb[:], rhs=w2lo_sb[:, 0:8],
                start=True, stop=True,
            )

            # ---- pointwise prep: jf = floor(x), qf = floor(x/2) ----
            x_pw = pw.tile([P, COLS], f32, tag="x")
            nc.sync.dma_start(out=x_pw[:], in_=x_view)
            jf_pw = pw.tile([P, COLS], bf16, tag="jf")
            r_pw = tmp.tile([P, COLS], f32, tag="ta", name="prep_r")
            nc.vector.tensor_scalar(
                r_pw[:], x_pw[:], 8388608.0, -8388608.0, Alu.add, Alu.add
            )
            d_pw = tmp.tile([P, COLS], f32, tag="tb", name="prep_d")
            nc.vector.tensor_tensor(
                out=d_pw[:], in0=r_pw[:], in1=x_pw[:], op=Alu.is_gt
            )
            nc.vector.tensor_tensor(
                out=jf_pw[:], in0=r_pw[:], in1=d_pw[:], op=Alu.subtract
            )
            hx_pw = tmp.tile([P, COLS], f32, tag="tc", name="prep_hx")
            nc.scalar.mul(hx_pw[:], x_pw[:], 0.5)
            t2_pw = tmp.tile([P, COLS], f32, tag="ta", name="prep_t2")
            nc.vector.tensor_scalar(
                t2_pw[:], hx_pw[:], 8388608.0, -8388608.0, Alu.add, Alu.add
            )
            d2_pw = tmp.tile([P, COLS], f32, tag="tb", name="prep_d2")
            nc.vector.tensor_tensor(
                out=d2_pw[:], in0=t2_pw[:], in1=hx_pw[:], op=Alu.is_gt
            )
            qf_pw = pw.tile([P, COLS], f32, tag="qf")
            nc.vector.tensor_tensor(
                out=qf_pw[:], in0=t2_pw[:], in1=d2_pw[:], op=Alu.subtract
            )
            v_pw = pw.tile([P, COLS], f32, tag="v")
            nc.vector.scalar_tensor_tensor(
                v_pw[:], qf_pw[:], -2.0, x_pw[:], Alu.mult, Alu.add
            )
            # r = jf - 2 qf  (0/1)
            rr_pw = pw.tile([P, COLS], f32, tag="rr")
            nc.vector.scalar_tensor_tensor(
                rr_pw[:], qf_pw[:], -2.0, jf_pw[:], Alu.mult, Alu.add
            )

            nc.sync.dma_start(
                out=jf_dram.ap().rearrange("(p f) -> p f", p=P), in_=jf_pw[:]
            )
            jf_view = jf_dram.ap().rearrange(
                "(s tp cc) -> s tp cc", s=NSLOT, tp=NCHUNK // 2
            )

            g_all = pw.tile([P, 2, 4, 2, TW], f32, tag="gall")
            g_view = g_dram_s.ap().rearrange(
                "u m (e h c) -> m u e (h c)", e=2, h=2
            )

            # ---- chunk loop ----
            for t in range(NCHUNK):
                if t == 9:
                    # first half of the units is stored; stream those loads
                    for cd in range(2):
                        for k in range(4):
                            for s in range(4):
                                nc.sync.dma_start(
                                    out=g_all[32 * s : 32 * s + 16, cd, k, :, :],
                                    in_=g_view[8 * s + 4 * cd + k, 0:8],
                                )
                j_pk = j_bufs[(t // 2) % 2]
                if t % 2 == 0:
                    nc.sync.dma_start(
                        out=j_pk[1:5, :], in_=jf_view[:, t // 2]
                    )
                    # consolidator for the jf-load semaphore
                    nc.tensor.matmul(
                        out=ps1_bufs[0][:, 0:8], lhsT=w1_sb[:],
                        rhs=j_pk[:, 0:8], start=True, stop=True,
                    )
                for tau in range(TPC):
                    b, h = tau // 2, tau % 2
                    tg = TPC * t + tau
                    q = tg // 2  # = 4 t + b
                    ps1 = ps1_bufs[tg % 4]
                    nc.tensor.matmul(
                        out=ps1[:],
                        lhsT=w1_sb[:],
                        rhs=j_pk[
                            :,
                            (t % 2) * CH + tau * TW : (t % 2) * CH
                            + (tau + 1) * TW,
                        ],
                        start=True,
                        stop=True,
                    )
                    s_sb = s_bufs[tg % 8]
                    if _eng_of(t, b) == "act":
                        nc.scalar.sign(s_sb[:], ps1[:])  # {-1, +1}
                    else:
                        nc.vector.tensor_scalar(
                            s_sb[:], ps1[:], 0.0, 2.0, Alu.is_ge, Alu.mult
                        )  # {0, 2}
                    ps2 = ps2_bufs[q % 2]
                    nc.tensor.matmul(
                        out=ps2[:, h * TW : (h + 1) * TW],
                        lhsT=w2hi_sb[:], rhs=s_sb[:],
                        start=True, stop=False,
                    )
                    nc.tensor.matmul(
                        out=ps2[:, h * TW : (h + 1) * TW],
                        lhsT=w2lo_sb[:], rhs=s_sb[:],
                        start=False, stop=True,
                    )
                    if h == 1:
                        gcp = gcp_full[(q // 2) % 2]
                        dstc = gcp[:, (q % 2) * 2 * TW : (q % 2 + 1) * 2 * TW]
                        if (q * 3) % 5 < 3:
                            nc.scalar.copy(out=dstc, in_=ps2[:])
                        else:
                            nc.vector.tensor_copy(out=dstc, in_=ps2[:])
                    if tau % 4 == 3:
                        U = tg // 4
                        nc.gpsimd.dma_start(
                            out=g_dram_s.ap()[U], in_=gcp_full[U % 2][:]
                        )

            # ---- remaining G loads (u >= 8) ----
            for cd in range(2):
                for k in range(4):
                    for s in range(4):
                        nc.sync.dma_start(
                            out=g_all[32 * s + 16 : 32 * s + 32, cd, k, :, :],
                            in_=g_view[8 * s + 4 * cd + k, 8:16],
                        )

            # ---- dual Horner: out = HC(v) + r * HD(v), + gamma on ACT rows --
            v2_pw = pw.tile([P, COLS], f32, tag="v2")
            nc.scalar.square(v2_pw[:], v_pw[:])
            hres = []
            for cd in range(2):
                gk = [
                    g_all[:, cd, k].rearrange("p h c -> p (h c)")
                    for k in range(4)
                ]
                g2c = tmp.tile([P, COLS], f32, tag="ta", name=f"g2c{cd}")
                nc.vector.tensor_scalar(
                    g2c[:], gk[2], gam_sb[:, 4 * cd + 2 : 4 * cd + 3], None,
                    Alu.add,
                )
                g3c = tmp.tile([P, COLS], f32, tag="tb", name=f"g3c{cd}")
                nc.vector.tensor_scalar(
                    g3c[:], gk[3], gam_sb[:, 4 * cd + 3 : 4 * cd + 4], None,
                    Alu.add,
                )
                v1t = tmp.tile([P, COLS], f32, tag="tc", name=f"v1t{cd}")
                nc.vector.tensor_tensor(
                    out=v1t[:], in0=g2c[:], in1=v2_pw[:], op=Alu.mult
                )
                v2t = tmp.tile([P, COLS], f32, tag="td", name=f"v2t{cd}")
                nc.vector.tensor_tensor(
                    out=v2t[:], in0=g3c[:], in1=v2_pw[:], op=Alu.mult
                )
                pacc = tmp.tile([P, COLS], f32, tag="ta", name=f"pacc{cd}")
                nc.vector.scalar_tensor_tensor(
                    pacc[:], v1t[:], gam_sb[:, 4 * cd : 4 * cd + 1], gk[0],
                    Alu.add, Alu.add,
                )
                qacc = tmp.tile([P, COLS], f32, tag="tb", name=f"qacc{cd}")
                nc.vector.scalar_tensor_tensor(
                    qacc[:], v2t[:], gam_sb[:, 4 * cd + 1 : 4 * cd + 2], gk[1],
                    Alu.add, Alu.add,
                )
                v3t = tmp.tile([P, COLS], f32, tag="tc", name=f"v3t{cd}")
                nc.vector.tensor_tensor(
                    out=v3t[:], in0=qacc[:], in1=v_pw[:], op=Alu.mult
                )
                hr = tmp.tile([P, COLS], f32, tag="td", name=f"hr{cd}")
                nc.vector.tensor_tensor(
                    out=hr[:], in0=pacc[:], in1=v3t[:], op=Alu.add
                )
                hres.append(hr)
            rd = tmp.tile([P, COLS], f32, tag="ta", name="rd")
            nc.vector.tensor_tensor(
                out=rd[:], in0=hres[1][:], in1=rr_pw[:], op=Alu.mult
            )
            res = tmp.tile([P, COLS], f32, tag="tb", name="res")
            nc.vector.tensor_tensor(
                out=res[:], in0=hres[0][:], in1=rd[:], op=Alu.add
            )
            nc.sync.dma_start(out=out_view, in_=res[:])

    nc.compile()
    return nc


def get_program():
    if "prog" not in _PROG_CACHE:
        _PROG_CACHE["prog"] = _build_program()
    return _PROG_CACHE["prog"]


def make_in_maps(x: np.ndarray, coefs: np.ndarray):
    _, cpack = _tables(coefs)
    shards = np.asarray(x, np.float32).reshape(N_CORES, N)
    return [
        {"x": shards[i].copy(), "cpack": cpack}
        for i in range(N_CORES)
    ]


def kernel(x, coefs, knot_vector=None, _trace: bool = False):
    from concourse.bass_utils import run_bass_kernel_spmd

    nc = get_program()
    in_maps = make_in_maps(x, coefs)
    res = run_bass_kernel_spmd(nc, in_maps, list(range(N_CORES)), trace=_trace)
    out = np.concatenate([r["out"] for r in res.results])
    if _trace:
        return out, res
    return out



# revision 17
# speedup vs baseline: 1.0968x; 1.0184x over previous
"""Cubic B-spline evaluation (uniform knots) on 8 Trainium2 NeuronCores.

v2: j = 2q + r split.  On segment j the spline is a cubic in v = x - 2q:
  out = HC(v) + r * HD(v),  HC = sum_k c_k[q] v^k,  HD = sum_k d_k[q] v^k
with 32-entry tables c, d (host-derived from coefs).  Table lookups become
step sums over 32 thresholds 1{q >= i} = 1{j >= 2i}, built as a K=5 bf16
matmul over 4 point-slots packed into 128 partitions (32 rows each), an
indicator pass (ScalarE Sign / VectorE is_ge), and a contraction with bf16
hi+lo difference weights.  Coefficient octets stream through DRAM scratch
into pointwise layout; a dual Horner finishes.

Layout (per core, N = 131072 = 4 slots x 32768):
  pointwise: x_pw[p, f] = x[1024 p + f]; p = 32 s + q, q = 2 t + b
  tiles: 64 x 512 cols; chunk t in [0,16) x tau in [0,4); tg = 4t + tau;
         q = tg//2, h = tg%2; unit U = q//2 = t, e = q%2
  g_all[p, cd, k, h, c]: coef k of table cd for point (p, f = 512 h + c)
"""

import sys

sys.path.insert(0, "/opt/trn_rl_repo")

import numpy as np

N_TOTAL = 1_048_576
N_CORES = 8
N = N_TOTAL // N_CORES  # 131072 points per core
P = 128
COLS = N // P  # 1024
TW = 512
NCHUNK = 16
TPC = 4
CH = TPC * TW  # 4096
NSLOT = 4
SLOTN = N // NSLOT  # 32768


def _tables(coefs: np.ndarray):
    import ml_dtypes

    c = np.zeros(67, np.float64)
    c[3:] = np.asarray(coefs, np.float64)
    jj = np.arange(64)
    a0 = (c[jj] + 4 * c[jj + 1] + c[jj + 2]) / 6
    a1 = (c[jj + 2] - c[jj]) / 2
    a2 = (c[jj] - 2 * c[jj + 1] + c[jj + 2]) / 2
    a3 = (c[jj + 3] - c[jj] + 3 * c[jj + 1] - 3 * c[jj + 2]) / 6
    A = np.stack([a0, a1, a2, a3], 1)  # [64, 4] coeffs in u = x - j

    # rebase odd segments to v = u + 1 (v = x - 2q)
    B = A.copy()
    r1 = jj % 2 == 1
    B[r1, 0] = A[r1, 0] - A[r1, 1] + A[r1, 2] - A[r1, 3]
    B[r1, 1] = A[r1, 1] - 2 * A[r1, 2] + 3 * A[r1, 3]
    B[r1, 2] = A[r1, 2] - 3 * A[r1, 3]
    B[r1, 3] = A[r1, 3]
    C = B[0::2]  # [32, 4]
    D = B[1::2] - B[0::2]  # [32, 4]

    # halved step-difference weights (unified sign/{0,2} convention)
    WC = C.copy()
    WC[1:] -= C[:-1]
    WD = D.copy()
    WD[1:] -= D[:-1]
    Wp = np.concatenate([WC, WD], 1) * 0.5  # [32, 8]: col 4 cd + k
    gamma_k = Wp.sum(0).astype(np.float32)  # [8]

    # MM1 lhsT [5, 128]: col m = 32 s + i -> psum = jf_s - thr_i
    w1 = np.zeros((5, 128), np.float64)
    thr = np.empty(32)
    thr[0] = -1.0
    thr[1:] = 2.0 * np.arange(1, 32) - 0.5
    for s in range(4):
        w1[1 + s, 32 * s : 32 * s + 32] = 1.0
        w1[0, 32 * s : 32 * s + 32] = -thr
    # MM2 lhsT [128, 32]: row m = 32 s' + i, col 8 s + 4 cd + k
    w2 = np.zeros((128, 32), np.float64)
    for s in range(4):
        w2[32 * s : 32 * s + 32, 8 * s : 8 * s + 8] = Wp
    bf = ml_dtypes.bfloat16
    w2hi = w2.astype(bf)
    w2lo = (w2 - w2hi.astype(np.float64)).astype(bf)
    # pack all constants into one [128, 208] bf16 tensor:
    #   cols   0:128  rows 0:5   w1
    #   cols 128:160  w2hi, cols 160:192  w2lo
    #   cols 192:208  gamma (f32 bit-packed as bf16 pairs)
    pack = np.zeros((128, 208), bf)
    pack[0:5, 0:128] = w1.astype(bf)
    pack[:, 128:160] = w2hi
    pack[:, 160:192] = w2lo
    gvec = _gamma_vec(gamma_k)  # [128, 8] f32
    pack[:, 192:208] = gvec.astype(np.float32).view(np.uint16).view(bf)
    return A, pack


def _eng_of(t: int, b: int) -> str:
    return "act" if (4 * t + b) % 5 < 3 else "dve"


def _gamma_vec(gamma_k: np.ndarray) -> np.ndarray:
    g = np.zeros((P, 8), np.float32)
    for p in range(P):
        q = p % 32  # q = 2 t + b  (TPC = 4: two pairs per chunk)
        if _eng_of(q // 2, q % 2) == "act":
            g[p] = gamma_k
    return g


_PROG_CACHE: dict = {}


def _build_program():
    import concourse.bacc as bacc
    import concourse.mybir as mybir
    from concourse.tile import TileContext

    f32 = mybir.dt.float32
    bf16 = mybir.dt.bfloat16
    Alu = mybir.AluOpType

    nc = bacc.Bacc("TRN2", debug=False)

    x_dram = nc.dram_tensor("x", [N], f32, kind="ExternalInput")
    cpack_dram = nc.dram_tensor("cpack", [P, 208], bf16, kind="ExternalInput")
    out_dram = nc.dram_tensor("out", [N], f32, kind="ExternalOutput")
    jf_dram = nc.dram_tensor("jf_scratch", [N], bf16, kind="Internal")
    g_dram_s = nc.dram_tensor(
        "g_scratch", [16, 2, 32, 2 * TW], f32, kind="Internal"
    )

    x_view = x_dram.ap().rearrange("(p f) -> p f", p=P)
    out_view = out_dram.ap().rearrange("(p f) -> p f", p=P)

    with TileContext(nc) as tc:
        with (
            tc.tile_pool(name="const", bufs=1) as cpool,
            tc.tile_pool(name="pw", bufs=1) as pw,
            tc.tile_pool(name="tmp", bufs=6) as tmp,
            tc.tile_pool(name="sind", bufs=1) as spool,
            tc.tile_pool(name="gcp", bufs=1) as gcpool,
            tc.tile_pool(name="psum1", bufs=1, space="PSUM") as pp1,
            tc.tile_pool(name="psum2", bufs=1, space="PSUM") as pp2,
        ):
            # ---- constants: one packed DMA; ones rows via memset ----
            cpk = cpool.tile([P, 208], bf16, tag="cpk")
            nc.sync.dma_start(out=cpk[:], in_=cpack_dram.ap())
            w1_sb = cpk[0:5, 0:128]
            w2hi_sb = cpk[:, 128:160]
            w2lo_sb = cpk[:, 160:192]
            gam_sb = cpk[:, 192:208].bitcast(f32)
            j_bufs = []
            for bi in range(2):
                jb = cpool.tile(
                    [5, 2 * CH], bf16, tag=f"jbuf{bi}", name=f"jbuf{bi}"
                )
                nc.vector.memset(jb[0:1, :], 1.0)
                j_bufs.append(jb)

            ps1_bufs = [
                pp1.tile([P, TW], f32, tag=f"s1_{i}", name=f"ps1f{i}")
                for i in range(4)
            ]
            ps2_bufs = [
                pp2.tile([32, 2 * TW], f32, tag=f"s2_{i}", name=f"ps2f{i}")
                for i in range(2)
            ]
            s_bufs = [
                spool.tile([P, TW], bf16, tag=f"sb_{i}", name=f"sbf{i}")
                for i in range(8)
            ]
            gcp_full = [
                gcpool.tile([32, 4 * TW], f32, tag=f"gc_{i}", name=f"gcpf{i}")
                for i in range(2)
            ]

            # dummies: absorb constant-load DMA sems into the PE vector clock
            pdum = ps1_bufs[0]
            nc.tensor.matmul(
                out=pdum[:, 0:8], lhsT=w1_sb[:], rhs=w1_sb[:, 0:8],
                start=True, stop=True,
            )
            nc.tensor.matmul(
                out=pdum[0:32, 0:8], lhsT=w2hi_sb[:], rhs=w2hi_sb[:, 0:8],
                start=True, stop=True,
            )
            nc.tensor.matmul(
                out=pdum[0:32, 0:8], lhsT=w2lo_sb[:], rhs=w2lo_sb[:, 0:8],
                start=True, stop=True,
            )

            # ---- pointwise prep: jf = floor(x), qf = floor(x/2) ----
            x_pw = pw.tile([P, COLS], f32, tag="x")
            nc.sync.dma_start(out=x_pw[:], in_=x_view)
            jf_pw = pw.tile([P, COLS], bf16, tag="jf")
            r_pw = tmp.tile([P, COLS], f32, tag="ta", name="prep_r")
            nc.vector.tensor_scalar(
                r_pw[:], x_pw[:], 8388608.0, -8388608.0, Alu.add, Alu.add
            )
            d_pw = tmp.tile([P, COLS], f32, tag="tb", name="prep_d")
            nc.vector.tensor_tensor(
                out=d_pw[:], in0=r_pw[:], in1=x_pw[:], op=Alu.is_gt
            )
            nc.vector.tensor_tensor(
                out=jf_pw[:], in0=r_pw[:], in1=d_pw[:], op=Alu.subtract
            )
            hx_pw = tmp.tile([P, COLS], f32, tag="tc", name="prep_hx")
            nc.scalar.mul(hx_pw[:], x_pw[:], 0.5)
            t2_pw = tmp.tile([P, COLS], f32, tag="ta", name="prep_t2")
            nc.vector.tensor_scalar(
                t2_pw[:], hx_pw[:], 8388608.0, -8388608.0, Alu.add, Alu.add
            )
            d2_pw = tmp.tile([P, COLS], f32, tag="tb", name="prep_d2")
            nc.vector.tensor_tensor(
                out=d2_pw[:], in0=t2_pw[:], in1=hx_pw[:], op=Alu.is_gt
            )
            qf_pw = pw.tile([P, COLS], f32, tag="qf")
            nc.vector.tensor_tensor(
                out=qf_pw[:], in0=t2_pw[:], in1=d2_pw[:], op=Alu.subtract
            )
            v_pw = pw.tile([P, COLS], f32, tag="v")
            nc.vector.scalar_tensor_tensor(
                v_pw[:], qf_pw[:], -2.0, x_pw[:], Alu.mult, Alu.add
            )
            # r = jf - 2 qf  (0/1)
            rr_pw = pw.tile([P, COLS], f32, tag="rr")
            nc.vector.scalar_tensor_tensor(
                rr_pw[:], qf_pw[:], -2.0, jf_pw[:], Alu.mult, Alu.add
            )

            nc.sync.dma_start(
                out=jf_dram.ap().rearrange("(p f) -> p f", p=P), in_=jf_pw[:]
            )
            jf_view = jf_dram.ap().rearrange(
                "(s tp cc) -> s tp cc", s=NSLOT, tp=NCHUNK // 2
            )

            g_all = pw.tile([P, 2, 4, 2, TW], f32, tag="gall")
            g_view = g_dram_s.ap().rearrange(
                "u e (s m) hc -> s (u e) m hc", s=4
            )

            # ---- chunk loop ----
            for t in range(NCHUNK):
                if t == 9:
                    # first half of the units is stored; stream those loads
                    for s in range(4):
                        nc.sync.dma_start(
                            out=g_all[32 * s : 32 * s + 16],
                            in_=g_view[s, 0:16],
                        )
                j_pk = j_bufs[(t // 2) % 2]
                if t % 2 == 0:
                    nc.sync.dma_start(
                        out=j_pk[1:5, :], in_=jf_view[:, t // 2]
                    )
                    # consolidator for the jf-load semaphore
                    nc.tensor.matmul(
                        out=ps1_bufs[0][:, 0:8], lhsT=w1_sb[:],
                        rhs=j_pk[:, 0:8], start=True, stop=True,
                    )
                for tau in range(TPC):
                    b, h = tau // 2, tau % 2
                    tg = TPC * t + tau
                    q = tg // 2  # = 4 t + b
                    ps1 = ps1_bufs[tg % 4]
                    nc.tensor.matmul(
                        out=ps1[:],
                        lhsT=w1_sb[:],
                        rhs=j_pk[
                            :,
                            (t % 2) * CH + tau * TW : (t % 2) * CH
                            + (tau + 1) * TW,
                        ],
                        start=True,
                        stop=True,
                    )
                    s_sb = s_bufs[tg % 8]
                    if _eng_of(t, b) == "act":
                        nc.scalar.sign(s_sb[:], ps1[:])  # {-1, +1}
                    else:
                        nc.vector.tensor_scalar(
                            s_sb[:], ps1[:], 0.0, 2.0, Alu.is_ge, Alu.mult
                        )  # {0, 2}
                    ps2 = ps2_bufs[q % 2]
                    nc.tensor.matmul(
                        out=ps2[:, h * TW : (h + 1) * TW],
                        lhsT=w2hi_sb[:], rhs=s_sb[:],
                        start=True, stop=False,
                    )
                    nc.tensor.matmul(
                        out=ps2[:, h * TW : (h + 1) * TW],
                        lhsT=w2lo_sb[:], rhs=s_sb[:],
                        start=False, stop=True,
                    )
                    if h == 1:
                        gcp = gcp_full[(q // 2) % 2]
                        dstc = gcp[:, (q % 2) * 2 * TW : (q % 2 + 1) * 2 * TW]
                        if (q * 3) % 5 < 3:
                            nc.scalar.copy(out=dstc, in_=ps2[:])
                        else:
                            nc.vector.tensor_copy(out=dstc, in_=ps2[:])
                    if tau % 4 == 3:
                        U = tg // 4
                        nc.gpsimd.dma_start(
                            out=g_dram_s.ap()[U].rearrange(
                                "e m hc -> m e hc"
                            ),
                            in_=gcp_full[U % 2][:].rearrange(
                                "m (e hc) -> m e hc", e=2
                            ),
                        )

            # ---- remaining G loads (u >= 8) ----
            for s in range(4):
                nc.sync.dma_start(
                    out=g_all[32 * s + 16 : 32 * s + 32],
                    in_=g_view[s, 16:32],
                )

            # ---- dual Horner: out = HC(v) + r * HD(v), + gamma on ACT rows --
            v2_pw = pw.tile([P, COLS], f32, tag="v2")
            nc.scalar.square(v2_pw[:], v_pw[:])
            hres = []
            for cd in range(2):
                gk = [
                    g_all[:, cd, k].rearrange("p h c -> p (h c)")
                    for k in range(4)
                ]
                g2c = tmp.tile([P, COLS], f32, tag="ta", name=f"g2c{cd}")
                nc.vector.tensor_scalar(
                    g2c[:], gk[2], gam_sb[:, 4 * cd + 2 : 4 * cd + 3], None,
                    Alu.add,
                )
                g3c = tmp.tile([P, COLS], f32, tag="tb", name=f"g3c{cd}")
                nc.vector.tensor_scalar(
                    g3c[:], gk[3], gam_sb[:, 4 * cd + 3 : 4 * cd + 4], None,
                    Alu.add,
                )
                v1t = tmp.tile([P, COLS], f32, tag="tc", name=f"v1t{cd}")
                nc.vector.tensor_tensor(
                    out=v1t[:], in0=g2c[:], in1=v2_pw[:], op=Alu.mult
                )
                v2t = tmp.tile([P, COLS], f32, tag="td", name=f"v2t{cd}")
                nc.vector.tensor_tensor(
                    out=v2t[:], in0=g3c[:], in1=v2_pw[:], op=Alu.mult
                )
                pacc = tmp.tile([P, COLS], f32, tag="ta", name=f"pacc{cd}")
                nc.vector.scalar_tensor_tensor(
                    pacc[:], v1t[:], gam_sb[:, 4 * cd : 4 * cd + 1], gk[0],
                    Alu.add, Alu.add,
                )
                qacc = tmp.tile([P, COLS], f32, tag="tb", name=f"qacc{cd}")
                nc.vector.scalar_tensor_tensor(
                    qacc[:], v2t[:], gam_sb[:, 4 * cd + 1 : 4 * cd + 2], gk[1],
                    Alu.add, Alu.add,
                )
                v3t = tmp.tile([P, COLS], f32, tag="tc", name=f"v3t{cd}")
                nc.vector.tensor_tensor(
                    out=v3t[:], in0=qacc[:], in1=v_pw[:], op=Alu.mult
                )
                hr = tmp.tile([P, COLS], f32, tag="td", name=f"hr{cd}")
                nc.vector.tensor_tensor(
                    out=hr[:], in0=pacc[:], in1=v3t[:], op=Alu.add
                )
                hres.append(hr)
            rd = tmp.tile([P, COLS], f32, tag="ta", name="rd")
            nc.vector.tensor_tensor(
                out=rd[:], in0=hres[1][:], in1=rr_pw[:], op=Alu.mult
            )
            res = tmp.tile([P, COLS], f32, tag="tb", name="res")
            nc.vector.tensor_tensor(
                out=res[:], in0=hres[0][:], in1=rd[:], op=Alu.add
            )
            nc.sync.dma_start(out=out_view, in_=res[:])

    nc.compile()
    return nc


def get_program():
    if "prog" not in _PROG_CACHE:
        _PROG_CACHE["prog"] = _build_program()
    return _PROG_CACHE["prog"]


def make_in_maps(x: np.ndarray, coefs: np.ndarray):
    _, cpack = _tables(coefs)
    shards = np.asarray(x, np.float32).reshape(N_CORES, N)
    return [
        {"x": shards[i].copy(), "cpack": cpack}
        for i in range(N_CORES)
    ]


def kernel(x, coefs, knot_vector=None, _trace: bool = False):
    from concourse.bass_utils import run_bass_kernel_spmd

    nc = get_program()
    in_maps = make_in_maps(x, coefs)
    res = run_bass_kernel_spmd(nc, in_maps, list(range(N_CORES)), trace=_trace)
    out = np.concatenate([r["out"] for r in res.results])
    if _trace:
        return out, res
    return out

